# revision 60
# baseline (speedup 1.0000x reference)
"""Trainium2 Bass kernel for the RNN-T JointNetwork problem.

Computes log_softmax(tanh(cat(enc, pred)) @ W.T + b) over the vocab dim
for logits of shape [B=4, T=200, U=50, V=1024].  Data-parallel over the
800 flattened (b,t) rows, 100 per core; 5000 output rows per core.

Measured story (traces on trn2): the kernel is a race to the output-DMA
floor.  The 16 shared DMA engines saturate at ~25 GB/s each for >=2KB
packets (~395 GB/s aggregate); per core the output is what dominates.
Changes vs the first working kernel (93.5us -> ~64us):

1. bf16 output (halves the write floor from ~55us to ~27us; the f32
   version measured the engines at 101% busy).  Host upcasts to f32
   after the gather.  The extra rounding keeps rel err ~5e-3, well
   under the 2e-2 gate.

2. Bias fixed + cheaper: the old kernel added b twice (pred projection
   AND comb bias row, a 0.031 abs error hidden by the tolerance).  Now
   b enters the exp/lse path via the ACT bias operand (per-partition
   bT column; v is on partitions there) and the x path only via the
   comb bias row.  Also deletes 8 K=1 bias matmuls.

3. Host-side chunk-major input layouts; ALL bulk loads ride ONE ring
   (sync) in priority order: enc, pred, 8 single-chunk W pieces, hot.
   Per-ring FIFO is the only ordering the DMA engines respect -- a
   bulk transfer on another ring steals engine slots from this one.
   enc/pred ship as bf16 (tanh still computed on device).  W streaming
   lets projections start as soon as piece 0 + tanh land (~12us).

4. Projection tail (S-contraction, transposes, comb copies) interleaved
   into the chunk loop with a 1-chunk lag so the in-order PE stream
   never stalls on cross-engine dependencies.

5. lse lands in the matmul, not the copies: comb row 51 = -1, hot row
   51 = runtime lse values, so the one-hot matmul emits x - lse and
   the PSUM->SBUF casts stay plain (a per-partition bias operand on the
   copies measured +15-20% copy time and made the loop copy-bound).
   The [t,u] -> [r//128, r%128] repack is done by TWO parallel one-hop
   SBUF->SBUF DMAs split at r = 3200 = LCM(50,128): 64 t-rows map to
   exactly 25 k-columns, so each piece's destination is one contiguous
   single-partition run (hot's k-dim is padded to 50 for the tail).

6. Main loop: per tile two 512-col matmuls (same hot stationary) into
   a 2-bank f32 PSUM tile (bufs=4), one full-tile cast to bf16
   alternating DVE / Scalar-ACT (the only engines that read PSUM),
   output DMA issue alternating Sync / GpSimd.  outs bufs=10 matters:
   the DMA-completion latency needs ~3 tiles of cover per queue or the
   copies stall on buffer recycle (outs=6 measured +6us).

Known-idle remainders: ~7us fixed NEFF startup/barrier, ~6us teardown,
~2.4us cold-queue latency on the lse hop, and the HAM clock governor
(k=4 vs k=8 duty cycling) which decides whether the projection phase
and early loop run at ~2/3 speed -- warm-up matmuls proved counter-
productive (they spend the k=8 budget before the loop needs it).
"""

import numpy as np

import concourse.bass as bass
import concourse.bacc as bacc
import concourse.tile as tile
from concourse import mybir
from concourse.bass_utils import run_bass_kernel_spmd

# Problem shapes (hardcoded per contract).
B, T, U, D, V = 4, 200, 50, 512, 1024
N_CORES = 8
BT = B * T                     # 800 flattened (b,t) rows
TPC = BT // N_CORES            # 100 (b,t) rows per core
ROWS = TPC * U                 # 5000 output rows per core
P = 128
NT = (ROWS + P - 1) // P       # 40 row-tiles per core
DC = D // P                    # 4 contraction chunks of 128 for D=512
NVC = V // P                   # 8 vocab chunks of 128
NPIECE = 8                     # W DMA pieces, 1 vocab chunk each
TU = TPC + U                   # 150: t and u stacked on the free dim
BIAS_ROW = 50                  # comb partition holding the bias row
LSE_ROW = 51                   # comb partition holding all -1 (lse)
ENC_BASE = 64                  # comb partition where the enc window starts
ENC_WIN_B = 64                 # comb_B enc window starts at t=64
A_TILES = 25                   # tiles 0..24 use comb_A (t span <= 63)
N_WARM = 2                     # dummy matmuls to wake the PE clock gate

f32 = mybir.dt.float32
bf16 = mybir.dt.bfloat16

TRACE = False
LAST_RESULT = None

_CACHE = {}


def _patch_act_tables():
    """Pin Exp/Ln/Identity to the one table set containing all three, so
    the activation table-load pass emits exactly one load."""
    if getattr(bacc, "_joint_act_patch", False):
        return
    orig = bacc.get_activation_tables

    def patched(arch):
        t = dict(orig(arch))
        keep = "natural_log_exp_and_others"
        drop = {
            mybir.ActivationFunctionType.Exp,
            mybir.ActivationFunctionType.Ln,
            mybir.ActivationFunctionType.Identity,
        }
        for name, fns in t.items():
            if name != keep:
                t[name] = set(fns) - drop
        return t

    bacc.get_activation_tables = patched
    bacc._joint_act_patch = True


def _build_hot():
    """Per-tile [128,128] one-hot stationaries (moving-comb row selectors).

    hot[p, k, m]: output row r = 128k + m takes moving-comb partition p
    with weight 1 when p is its u-row (p = u(r)), the bias row
    (p = BIAS_ROW), or its t-row (p = ENC_BASE + t(r) - win0(k)).
    Row LSE_ROW is filled at runtime with the per-row lse values.
    Columns for r >= ROWS are all-zero.
    """
    r = np.arange(NT * P)
    valid = r < ROWS
    u = r % U
    t = r // U
    win0 = np.where((r // P) < A_TILES, 0, ENC_WIN_B)
    hot = np.zeros((P, NT * P), dtype=np.float32)
    hot[u[valid], r[valid]] = 1.0
    hot[BIAS_ROW, valid] = 1.0
    hot[(ENC_BASE + t - win0)[valid], r[valid]] = 1.0
    return np.ascontiguousarray(hot.reshape(P, NT, P))


def _build_program():
    import ml_dtypes

    _patch_act_tables()
    nc = bacc.Bacc("TRN2", target_bir_lowering=False, debug=False,
                   num_devices=N_CORES)

    encC = nc.dram_tensor("encC", [P, DC, TPC], bf16, kind="ExternalInput")
    predC = nc.dram_tensor("predC", [P, DC, U], bf16, kind="ExternalInput")
    wTc = nc.dram_tensor("wTc", [NPIECE, P, 2 * DC, P], bf16,
                         kind="ExternalInput")
    biasB = nc.dram_tensor("biasB", [1, V], bf16, kind="ExternalInput")
    biasT = nc.dram_tensor("biasT", [P, NVC], f32, kind="ExternalInput")
    out = nc.dram_tensor("out", [ROWS, V], bf16, kind="ExternalOutput")

    hot_dram = nc.inline_tensor(
        _build_hot().astype(ml_dtypes.bfloat16), name="hot")
    eye_bf_dram = nc.inline_tensor(
        np.eye(P, dtype=np.float32).astype(ml_dtypes.bfloat16), name="eyebf")
    negones_dram = nc.inline_tensor(
        np.full((1, V), -1.0, dtype=np.float32).astype(ml_dtypes.bfloat16),
        name="negones")

    Act = mybir.ActivationFunctionType
    PSUM = bass.MemorySpace.PSUM

    with tile.TileContext(nc) as tc:
        with (
            tc.tile_pool(name="consts", bufs=1) as consts,
            tc.tile_pool(name="outs", bufs=12) as outs,
        ):
            # ---- dummy ACT op first: binds the single table load before
            #      any HWDGE DMA is outstanding --------------------------
            dummy = consts.tile([1, 1], f32)
            nc.vector.memset(dummy[:], 0.0)
            nc.scalar.activation(dummy[:], dummy[:], Act.Identity)
            warm_mv = consts.tile([P, 512], bf16)
            nc.vector.memset(warm_mv[:], 0.0)

            # ---- input DMAs --------------------------------------------
            # Everything is host-pre-arranged so each DMA is one
            # contiguous run per partition.  ALL bulk loads go on the
            # sync ring in priority order -- the per-ring FIFO is the
            # only ordering the DMA engines respect, and a big transfer
            # on another ring steals engine slots from this one (v2 put
            # hot on the gpsimd ring first and it delayed pred/W by 5+us
            # and landed itself at 30us).
            enc_in = consts.tile([P, DC, TPC], bf16)
            nc.sync.dma_start(out=enc_in[:], in_=encC.ap())
            pred_in = consts.tile([P, DC, U], bf16)
            nc.sync.dma_start(out=pred_in[:], in_=predC.ap())
            wt = []
            for q in range(NPIECE):
                wq = consts.tile([P, 2 * DC, P], bf16, name=f"wt{q}")
                nc.sync.dma_start(out=wq[:], in_=wTc.ap()[q])
                wt.append(wq)
            # k-dim padded to 50 so the lse write splits at r = 3200
            # (= LCM(50,128)): columns 40..49 are never read.
            hot_sb = consts.tile([P, 50, P], bf16)
            nc.sync.dma_start(out=hot_sb[:, 0:NT, :], in_=hot_dram.ap())

            eye_bf = consts.tile([P, P], bf16)
            nc.gpsimd.dma_start(out=eye_bf[:], in_=eye_bf_dram.ap())
            bT_sb = consts.tile([P, NVC], f32)
            nc.gpsimd.dma_start(out=bT_sb[:], in_=biasT.ap())

            comb_A = consts.tile([P, V], bf16)
            comb_B = consts.tile([P, V], bf16)
            # zero the never-written partitions (52..63; 100..127 of B):
            # their one-hot weight is 0, but 0 x sbuf-garbage-NaN would
            # still poison the matmul.
            nc.vector.memset(comb_A[:], 0.0)
            nc.gpsimd.memset(comb_B[:], 0.0)
            # bias row: b enters the x path only here.  lse row: all -1;
            # hot row 51 carries the runtime lse values, so the one-hot
            # matmul emits x - lse directly and the copies stay plain.
            nc.gpsimd.dma_start(out=comb_A[BIAS_ROW:BIAS_ROW + 1, :],
                                in_=biasB.ap())
            nc.gpsimd.dma_start(out=comb_B[BIAS_ROW:BIAS_ROW + 1, :],
                                in_=biasB.ap())
            nc.gpsimd.dma_start(out=comb_A[LSE_ROW:LSE_ROW + 1, :],
                                in_=negones_dram.ap())
            nc.gpsimd.dma_start(out=comb_B[LSE_ROW:LSE_ROW + 1, :],
                                in_=negones_dram.ap())

            # ---- tanh without a second table set or a DVE divide -------
            # tanh(x) = 1 - 2/(1+e^2x);  1/(1+y) = exp(-ln(1+y))
            # single chain each: ACT cost is overhead-dominated, so fewer
            # bigger passes beat chunked ones.
            enc_bf = consts.tile([P, DC, TPC], bf16)
            pred_bf = consts.tile([P, DC, U], bf16)
            y1 = consts.tile([P, DC, TPC], f32)
            nc.scalar.activation(y1[:], enc_in[:], Act.Exp, scale=2.0)
            y2 = consts.tile([P, DC, TPC], f32)
            nc.scalar.activation(y2[:], y1[:], Act.Ln, bias=1.0)
            y3 = consts.tile([P, DC, TPC], f32)
            nc.scalar.activation(y3[:], y2[:], Act.Exp, scale=-1.0)
            nc.vector.tensor_scalar(enc_bf[:], y3[:], -2.0, 1.0,
                                    mybir.AluOpType.mult,
                                    mybir.AluOpType.add)
            p1 = consts.tile([P, DC, U], f32)
            nc.scalar.activation(p1[:], pred_in[:], Act.Exp, scale=2.0)
            p2 = consts.tile([P, DC, U], f32)
            nc.scalar.activation(p2[:], p1[:], Act.Ln, bias=1.0)
            p3 = consts.tile([P, DC, U], f32)
            nc.scalar.activation(p3[:], p2[:], Act.Exp, scale=-1.0)
            nc.vector.tensor_scalar(pred_bf[:], p3[:], -2.0, 1.0,
                                    mybir.AluOpType.mult,
                                    mybir.AluOpType.add)

            proj_sb = consts.tile([P, NVC, TU], bf16)
            expT_sb = consts.tile([P, NVC, TU], bf16)
            # lse_all padded to 128 partitions so the SBUF->SBUF repack
            # DMA moves exactly 128*50 = 50*128 elements (tail garbage
            # lands in unused hot k-columns >= 40).
            lse_all = consts.tile([P, U], bf16)
            nc.vector.memset(lse_all[:], 0.0)

            # ---- per-vocab-chunk: projT GEMM -> exp; S/transposes lag
            #      one chunk behind so the PE stream never stalls -------
            with (
                tc.tile_pool(name="psW", bufs=1, space=PSUM) as psW,
                tc.tile_pool(name="psA", bufs=2, space=PSUM) as psA,
                tc.tile_pool(name="psT", bufs=3, space=PSUM) as psT,
                tc.tile_pool(name="psB", bufs=1, space=PSUM) as psB,
            ):
                # PE warm-up while inputs stream: HAM un-throttles the PE
                # clock after ~4-5us of sustained activity, timed to land
                # the fast window on the projection phase.
                warm_ps = psW.tile([P, 512], f32)
                for w in range(N_WARM):
                    nc.tensor.matmul(warm_ps[:], warm_mv[:, 0:P],
                                     warm_mv[:], start=True, stop=True)

                s_ps = psB.tile([TPC, U], f32)

                def emit_tail(j):
                    # lse contraction: S = sum_j exp(e)_j . exp(p+b)_j
                    nc.tensor.matmul(s_ps[:], expT_sb[:, j, 0:TPC],
                                     expT_sb[:, j, TPC:TU],
                                     start=(j == 0), stop=(j == NVC - 1))
                    if j == NVC - 1:
                        # repack lse from [t, u] to r-order straight into
                        # hot partition 51.  Split at r = 3200 (64 t-rows
                        # = 25 k-columns; LCM(50,128) = 3200): each piece
                        # is ONE one-hop SBUF->SBUF DMA whose destination
                        # is a contiguous single-partition run, and the
                        # two pieces ride different queues in parallel.
                        # Tiles 0..24 (comb_A) unblock on piece A alone,
                        # so ln runs split too and piece A's DMA issues
                        # before the second ln half even starts.
                        nc.scalar.activation(lse_all[0:64, :],
                                             s_ps[0:64, :], Act.Ln)
                        nc.scalar.dma_start(
                            out=hot_sb[LSE_ROW:LSE_ROW + 1, 0:A_TILES, :],
                            in_=lse_all[0:64, :])
                        nc.scalar.activation(lse_all[64:TPC, :],
                                             s_ps[64:TPC, :], Act.Ln)
                        nc.sync.dma_start(
                            out=hot_sb[LSE_ROW:LSE_ROW + 1, A_TILES:50, :],
                            in_=lse_all[64:P, :])
                    vsl = slice(j * P, (j + 1) * P)
                    tr_e = psT.tile([TPC, P], bf16, name="tr_e", tag="tr",
                                    bufs=3)
                    nc.tensor.transpose(tr_e[:], proj_sb[:, j, 0:TPC],
                                        eye_bf[:])
                    tr_p = psT.tile([U, P], bf16, name="tr_p", tag="tr",
                                    bufs=3)
                    nc.tensor.transpose(tr_p[:], proj_sb[:, j, TPC:TU],
                                        eye_bf[:])
                    # GpSimd cannot read PSUM: DVE drains the transposes,
                    # gpsimd mirrors the shared pred rows SBUF->SBUF.
                    nc.vector.tensor_copy(comb_A[0:U, vsl], tr_p[:])
                    nc.gpsimd.tensor_copy(comb_B[0:U, vsl], comb_A[0:U, vsl])
                    nc.vector.tensor_copy(comb_A[ENC_BASE:P, vsl],
                                          tr_e[0:P - ENC_BASE, :])
                    nc.vector.tensor_copy(
                        comb_B[ENC_BASE:ENC_BASE + TPC - ENC_WIN_B, vsl],
                        tr_e[ENC_WIN_B:TPC, :])

                for j in range(NVC):
                    projT = psA.tile([P, TU], f32)
                    for c in range(DC):
                        nc.tensor.matmul(projT[:, 0:TPC],
                                         wt[j][:, c, :],
                                         enc_bf[:, c, :],
                                         start=(c == 0), stop=(c == DC - 1))
                    for c in range(DC):
                        nc.tensor.matmul(projT[:, TPC:TU],
                                         wt[j][:, DC + c, :],
                                         pred_bf[:, c, :],
                                         start=(c == 0), stop=(c == DC - 1))
                    # b enters the exp via the ACT bias operand (v is on
                    # partitions here, so bT[:, j] is the right column)
                    nc.scalar.activation(expT_sb[:, j, 0:TPC],
                                         projT[:, 0:TPC], Act.Exp)
                    nc.scalar.activation(expT_sb[:, j, TPC:TU],
                                         projT[:, TPC:TU], Act.Exp,
                                         bias=bT_sb[:, j:j + 1])
                    nc.vector.tensor_copy(proj_sb[:, j, :], projT[:])
                    if j >= 1:
                        emit_tail(j - 1)
                emit_tail(NVC - 1)

            # ---- main loop ---------------------------------------------
            # x tile = one one-hot matmul pass over comb (emits e+p+b);
            # the -lse lands in the PSUM->SBUF cast via the per-partition
            # scalar operand, so the matmuls depend only on hot + comb.
            with tc.tile_pool(name="psX", bufs=4, space=PSUM) as psX:
                for k in range(NT):
                    r0 = k * P
                    rows = min(P, ROWS - r0)
                    comb = comb_A if k < A_TILES else comb_B
                    x_ps = psX.tile([P, V], f32, tag="x")
                    for half in range(2):
                        sl = slice(half * 512, (half + 1) * 512)
                        nc.tensor.matmul(x_ps[:, sl], hot_sb[:, k, :],
                                         comb[:, sl], start=True, stop=True)
                    o = outs.tile([P, V], bf16)
                    if k % 2 == 0:
                        nc.vector.tensor_copy(o[:rows], x_ps[:rows])
                    else:
                        nc.scalar.activation(o[:rows], x_ps[:rows],
                                             Act.Identity)
                    eng = nc.sync if k % 2 == 0 else nc.gpsimd
                    eng.dma_start(out=out.ap()[r0:r0 + rows, :], in_=o[:rows])

    nc.compile()
    return nc


def kernel(enc_out, pred_out, W, b):
    global LAST_RESULT
    enc_out = np.asarray(enc_out, dtype=np.float32)
    pred_out = np.asarray(pred_out, dtype=np.float32)
    W = np.asarray(W, dtype=np.float32)
    b = np.asarray(b, dtype=np.float32)

    if "nc" not in _CACHE:
        _CACHE["nc"] = _build_program()
    nc = _CACHE["nc"]

    import ml_dtypes
    wT = np.ascontiguousarray(W.T)                              # [2D, V]
    # wTc[j, p, half*4+c, v'] = wT[512*half + 4p + c, 128j + v']
    wTc = np.ascontiguousarray(
        wT.reshape(2, P, DC, NPIECE, P)
          .transpose(3, 1, 0, 2, 4)
          .reshape(NPIECE, P, 2 * DC, P)).astype(ml_dtypes.bfloat16)
    bB = np.ascontiguousarray(b.reshape(1, V)).astype(ml_dtypes.bfloat16)
    bT = np.ascontiguousarray(b.reshape(NVC, P).T)              # [128, 8]
    enc_flat = enc_out.reshape(BT, D)                           # [800, 512]

    in_maps = []
    for c in range(N_CORES):
        bt0 = c * TPC
        b_idx = bt0 // T
        encCh = np.ascontiguousarray(
            enc_flat[bt0:bt0 + TPC].T.reshape(P, DC, TPC)).astype(
                ml_dtypes.bfloat16)
        predCh = np.ascontiguousarray(
            pred_out[b_idx].T.reshape(P, DC, U)).astype(ml_dtypes.bfloat16)
        in_maps.append({
            "encC": encCh,
            "predC": predCh,
            "wTc": wTc,
            "biasB": bB,
            "biasT": bT,
        })

    res = run_bass_kernel_spmd(nc, in_maps, core_ids=list(range(N_CORES)),
                               trace=TRACE)
    LAST_RESULT = res
    full = np.concatenate(
        [np.asarray(r["out"]) for r in res.results], axis=0)
    return full.astype(np.float32).reshape(B, T, U, V)


# revision 62
# speedup vs baseline: 1.0129x; 1.0129x over previous
"""Trainium2 Bass kernel for the RNN-T JointNetwork problem.

Computes log_softmax(tanh(cat(enc, pred)) @ W.T + b) over the vocab dim
for logits of shape [B=4, T=200, U=50, V=1024].  Data-parallel over the
800 flattened (b,t) rows, 100 per core; 5000 output rows per core.

Measured story (traces on trn2): the kernel is a race to the output-DMA
floor.  The 16 shared DMA engines saturate at ~25 GB/s each for >=2KB
packets (~395 GB/s aggregate); per core the output is what dominates.
Changes vs the first working kernel (93.5us -> ~64us):

1. bf16 output (halves the write floor from ~55us to ~27us; the f32
   version measured the engines at 101% busy).  Host upcasts to f32
   after the gather.  The extra rounding keeps rel err ~5e-3, well
   under the 2e-2 gate.

2. Bias fixed + cheaper: the old kernel added b twice (pred projection
   AND comb bias row, a 0.031 abs error hidden by the tolerance).  Now
   b enters the exp/lse path via the ACT bias operand (per-partition
   bT column; v is on partitions there) and the x path only via the
   comb bias row.  Also deletes 8 K=1 bias matmuls.

3. Host-side chunk-major input layouts; ALL bulk loads ride ONE ring
   (sync) in priority order: enc, pred, 8 single-chunk W pieces, hot.
   Per-ring FIFO is the only ordering the DMA engines respect -- a
   bulk transfer on another ring steals engine slots from this one.
   enc/pred ship as bf16 (tanh still computed on device).  W streaming
   lets projections start as soon as piece 0 + tanh land (~12us).

4. Projection tail (S-contraction, transposes, comb copies) interleaved
   into the chunk loop with a 1-chunk lag so the in-order PE stream
   never stalls on cross-engine dependencies.

5. lse lands in the matmul, not the copies: comb row 51 = -1, hot row
   51 = runtime lse values, so the one-hot matmul emits x - lse and
   the PSUM->SBUF casts stay plain (a per-partition bias operand on the
   copies measured +15-20% copy time and made the loop copy-bound).
   The [t,u] -> [r//128, r%128] repack is done by TWO parallel one-hop
   SBUF->SBUF DMAs split at r = 3200 = LCM(50,128): 64 t-rows map to
   exactly 25 k-columns, so each piece's destination is one contiguous
   single-partition run (hot's k-dim is padded to 50 for the tail).

6. Main loop: per tile two 512-col matmuls (same hot stationary) into
   a 2-bank f32 PSUM tile (bufs=4), one full-tile cast to bf16
   alternating DVE / Scalar-ACT (the only engines that read PSUM),
   output DMA issue alternating Sync / GpSimd.  outs bufs=10 matters:
   the DMA-completion latency needs ~3 tiles of cover per queue or the
   copies stall on buffer recycle (outs=6 measured +6us).

Known-idle remainders: ~7us fixed NEFF startup/barrier, ~6us teardown,
~2.4us cold-queue latency on the lse hop, and the HAM clock governor
(k=4 vs k=8 duty cycling) which decides whether the projection phase
and early loop run at ~2/3 speed -- warm-up matmuls proved counter-
productive (they spend the k=8 budget before the loop needs it).
"""

import numpy as np

import concourse.bass as bass
import concourse.bacc as bacc
import concourse.tile as tile
from concourse import mybir
from concourse.bass_utils import run_bass_kernel_spmd

# Problem shapes (hardcoded per contract).
B, T, U, D, V = 4, 200, 50, 512, 1024
N_CORES = 8
BT = B * T                     # 800 flattened (b,t) rows
TPC = BT // N_CORES            # 100 (b,t) rows per core
ROWS = TPC * U                 # 5000 output rows per core
P = 128
NT = (ROWS + P - 1) // P       # 40 row-tiles per core
DC = D // P                    # 4 contraction chunks of 128 for D=512
NVC = V // P                   # 8 vocab chunks of 128
NPIECE = 8                     # W DMA pieces, 1 vocab chunk each
TU = TPC + U                   # 150: t and u stacked on the free dim
BIAS_ROW = 50                  # comb partition holding the bias row
LSE_ROW = 51                   # comb partition holding all -1 (lse)
ENC_BASE = 64                  # comb partition where the enc window starts
ENC_WIN_B = 64                 # comb_B enc window starts at t=64
A_TILES = 25                   # tiles 0..24 use comb_A (t span <= 63)
N_WARM = 2                     # dummy matmuls to wake the PE clock gate

f32 = mybir.dt.float32
bf16 = mybir.dt.bfloat16

TRACE = False
LAST_RESULT = None

_CACHE = {}


def _patch_act_tables():
    """Pin Exp/Ln/Identity to the one table set containing all three, so
    the activation table-load pass emits exactly one load."""
    if getattr(bacc, "_joint_act_patch", False):
        return
    orig = bacc.get_activation_tables

    def patched(arch):
        t = dict(orig(arch))
        keep = "natural_log_exp_and_others"
        drop = {
            mybir.ActivationFunctionType.Exp,
            mybir.ActivationFunctionType.Ln,
            mybir.ActivationFunctionType.Identity,
        }
        for name, fns in t.items():
            if name != keep:
                t[name] = set(fns) - drop
        return t

    bacc.get_activation_tables = patched
    bacc._joint_act_patch = True


def _build_hot():
    """Per-tile [128,128] one-hot stationaries (moving-comb row selectors).

    hot[p, k, m]: output row r = 128k + m takes moving-comb partition p
    with weight 1 when p is its u-row (p = u(r)), the bias row
    (p = BIAS_ROW), or its t-row (p = ENC_BASE + t(r) - win0(k)).
    Row LSE_ROW is filled at runtime with the per-row lse values.
    Columns for r >= ROWS are all-zero.
    """
    r = np.arange(NT * P)
    valid = r < ROWS
    u = r % U
    t = r // U
    win0 = np.where((r // P) < A_TILES, 0, ENC_WIN_B)
    hot = np.zeros((P, NT * P), dtype=np.float32)
    hot[u[valid], r[valid]] = 1.0
    hot[BIAS_ROW, valid] = 1.0
    hot[(ENC_BASE + t - win0)[valid], r[valid]] = 1.0
    return np.ascontiguousarray(hot.reshape(P, NT, P))


def _build_program():
    import ml_dtypes

    _patch_act_tables()
    nc = bacc.Bacc("TRN2", target_bir_lowering=False, debug=False,
                   num_devices=N_CORES)

    encC = nc.dram_tensor("encC", [P, DC, TPC], bf16, kind="ExternalInput")
    predC = nc.dram_tensor("predC", [P, DC, U], bf16, kind="ExternalInput")
    wTc = nc.dram_tensor("wTc", [NPIECE, P, 2 * DC, P], bf16,
                         kind="ExternalInput")
    biasB = nc.dram_tensor("biasB", [1, V], bf16, kind="ExternalInput")
    biasT = nc.dram_tensor("biasT", [P, NVC], f32, kind="ExternalInput")
    out = nc.dram_tensor("out", [ROWS, V], bf16, kind="ExternalOutput")

    hot_dram = nc.inline_tensor(
        _build_hot().astype(ml_dtypes.bfloat16), name="hot")
    eye_bf_dram = nc.inline_tensor(
        np.eye(P, dtype=np.float32).astype(ml_dtypes.bfloat16), name="eyebf")
    negones_dram = nc.inline_tensor(
        np.full((1, V), -1.0, dtype=np.float32).astype(ml_dtypes.bfloat16),
        name="negones")

    Act = mybir.ActivationFunctionType
    PSUM = bass.MemorySpace.PSUM

    with tile.TileContext(nc) as tc:
        with (
            tc.tile_pool(name="consts", bufs=1) as consts,
            tc.tile_pool(name="outs", bufs=12) as outs,
        ):
            # ---- dummy ACT op first: binds the single table load before
            #      any HWDGE DMA is outstanding --------------------------
            dummy = consts.tile([1, 1], f32)
            nc.vector.memset(dummy[:], 0.0)
            nc.scalar.activation(dummy[:], dummy[:], Act.Identity)
            warm_mv = consts.tile([P, 512], bf16)
            nc.vector.memset(warm_mv[:], 0.0)

            # ---- input DMAs --------------------------------------------
            # Everything is host-pre-arranged so each DMA is one
            # contiguous run per partition.  ALL bulk loads go on the
            # sync ring in priority order -- the per-ring FIFO is the
            # only ordering the DMA engines respect, and a big transfer
            # on another ring steals engine slots from this one (v2 put
            # hot on the gpsimd ring first and it delayed pred/W by 5+us
            # and landed itself at 30us).
            enc_in = consts.tile([P, DC, TPC], bf16)
            nc.sync.dma_start(out=enc_in[:], in_=encC.ap())
            pred_in = consts.tile([P, DC, U], bf16)
            nc.sync.dma_start(out=pred_in[:], in_=predC.ap())
            wt = []
            for q in range(NPIECE):
                wq = consts.tile([P, 2 * DC, P], bf16, name=f"wt{q}")
                nc.sync.dma_start(out=wq[:], in_=wTc.ap()[q])
                wt.append(wq)
            # k-dim padded to 50 so the lse write splits at r = 3200
            # (= LCM(50,128)): columns 40..49 are never read.
            hot_sb = consts.tile([P, 50, P], bf16)
            nc.sync.dma_start(out=hot_sb[:, 0:NT, :], in_=hot_dram.ap())

            eye_bf = consts.tile([P, P], bf16)
            nc.gpsimd.dma_start(out=eye_bf[:], in_=eye_bf_dram.ap())
            bT_sb = consts.tile([P, NVC], f32)
            nc.gpsimd.dma_start(out=bT_sb[:], in_=biasT.ap())

            comb_A = consts.tile([P, V], bf16)
            comb_B = consts.tile([P, V], bf16)
            # zero the never-written partitions (52..63; 100..127 of B):
            # their one-hot weight is 0, but 0 x sbuf-garbage-NaN would
            # still poison the matmul.
            nc.vector.memset(comb_A[:], 0.0)
            nc.gpsimd.memset(comb_B[:], 0.0)
            # bias row: b enters the x path only here.  lse row: all -1;
            # hot row 51 carries the runtime lse values, so the one-hot
            # matmul emits x - lse directly and the copies stay plain.
            nc.gpsimd.dma_start(out=comb_A[BIAS_ROW:BIAS_ROW + 1, :],
                                in_=biasB.ap())
            nc.gpsimd.dma_start(out=comb_B[BIAS_ROW:BIAS_ROW + 1, :],
                                in_=biasB.ap())
            nc.gpsimd.dma_start(out=comb_A[LSE_ROW:LSE_ROW + 1, :],
                                in_=negones_dram.ap())
            nc.gpsimd.dma_start(out=comb_B[LSE_ROW:LSE_ROW + 1, :],
                                in_=negones_dram.ap())

            # ---- tanh without a second table set or a DVE divide -------
            # tanh(x) = 1 - 2/(1+e^2x);  1/(1+y) = exp(-ln(1+y))
            # single chain each: ACT cost is overhead-dominated, so fewer
            # bigger passes beat chunked ones.
            enc_bf = consts.tile([P, DC, TPC], bf16)
            pred_bf = consts.tile([P, DC, U], bf16)
            y1 = consts.tile([P, DC, TPC], f32)
            nc.scalar.activation(y1[:], enc_in[:], Act.Exp, scale=2.0)
            y2 = consts.tile([P, DC, TPC], f32)
            nc.scalar.activation(y2[:], y1[:], Act.Ln, bias=1.0)
            y3 = consts.tile([P, DC, TPC], f32)
            nc.scalar.activation(y3[:], y2[:], Act.Exp, scale=-1.0)
            nc.vector.tensor_scalar(enc_bf[:], y3[:], -2.0, 1.0,
                                    mybir.AluOpType.mult,
                                    mybir.AluOpType.add)
            p1 = consts.tile([P, DC, U], f32)
            nc.scalar.activation(p1[:], pred_in[:], Act.Exp, scale=2.0)
            p2 = consts.tile([P, DC, U], f32)
            nc.scalar.activation(p2[:], p1[:], Act.Ln, bias=1.0)
            p3 = consts.tile([P, DC, U], f32)
            nc.scalar.activation(p3[:], p2[:], Act.Exp, scale=-1.0)
            nc.vector.tensor_scalar(pred_bf[:], p3[:], -2.0, 1.0,
                                    mybir.AluOpType.mult,
                                    mybir.AluOpType.add)

            proj_sb = consts.tile([P, NVC, TU], bf16)
            expT_sb = consts.tile([P, NVC, TU], bf16)
            # lse_all padded to 128 partitions so the SBUF->SBUF repack
            # DMA moves exactly 128*50 = 50*128 elements (tail garbage
            # lands in unused hot k-columns >= 40).
            lse_all = consts.tile([P, U], bf16)
            nc.vector.memset(lse_all[:], 0.0)
            qwarm = consts.tile([1, 64], bf16)

            # ---- per-vocab-chunk: projT GEMM -> exp; S/transposes lag
            #      one chunk behind so the PE stream never stalls -------
            with (
                tc.tile_pool(name="psW", bufs=1, space=PSUM) as psW,
                tc.tile_pool(name="psA", bufs=2, space=PSUM) as psA,
                tc.tile_pool(name="psT", bufs=3, space=PSUM) as psT,
                tc.tile_pool(name="psB", bufs=1, space=PSUM) as psB,
            ):
                # PE warm-up while inputs stream: HAM un-throttles the PE
                # clock after ~4-5us of sustained activity, timed to land
                # the fast window on the projection phase.
                warm_ps = psW.tile([P, 512], f32)
                for w in range(N_WARM):
                    nc.tensor.matmul(warm_ps[:], warm_mv[:, 0:P],
                                     warm_mv[:], start=True, stop=True)

                s_ps = psB.tile([TPC, U], f32)

                def emit_tail(j):
                    if j in (2, 4, 6):
                        # keep the sync DMA queue's descriptor pipeline
                        # warm through the projection phase (the engine
                        # is idle here) so the lse hot-row write below
                        # doesn't pay ~1.5us of cold-queue latency.
                        nc.sync.dma_start(out=qwarm[:],
                                          in_=negones_dram.ap()[0:1, 0:64])
                    # lse contraction: S = sum_j exp(e)_j . exp(p+b)_j
                    nc.tensor.matmul(s_ps[:], expT_sb[:, j, 0:TPC],
                                     expT_sb[:, j, TPC:TU],
                                     start=(j == 0), stop=(j == NVC - 1))
                    if j == NVC - 1:
                        # repack lse from [t, u] to r-order straight into
                        # hot partition 51.  Split at r = 3200 (64 t-rows
                        # = 25 k-columns; LCM(50,128) = 3200): each piece
                        # is ONE one-hop SBUF->SBUF DMA whose destination
                        # is a contiguous single-partition run, and the
                        # two pieces ride different queues in parallel.
                        # Tiles 0..24 (comb_A) unblock on piece A alone,
                        # so ln runs split too and piece A's DMA issues
                        # before the second ln half even starts.
                        nc.scalar.activation(lse_all[0:64, :],
                                             s_ps[0:64, :], Act.Ln)
                        nc.scalar.dma_start(
                            out=hot_sb[LSE_ROW:LSE_ROW + 1, 0:A_TILES, :],
                            in_=lse_all[0:64, :])
                        nc.scalar.activation(lse_all[64:TPC, :],
                                             s_ps[64:TPC, :], Act.Ln)
                        nc.sync.dma_start(
                            out=hot_sb[LSE_ROW:LSE_ROW + 1, A_TILES:50, :],
                            in_=lse_all[64:P, :])
                    vsl = slice(j * P, (j + 1) * P)
                    tr_e = psT.tile([TPC, P], bf16, name="tr_e", tag="tr",
                                    bufs=3)
                    nc.tensor.transpose(tr_e[:], proj_sb[:, j, 0:TPC],
                                        eye_bf[:])
                    tr_p = psT.tile([U, P], bf16, name="tr_p", tag="tr",
                                    bufs=3)
                    nc.tensor.transpose(tr_p[:], proj_sb[:, j, TPC:TU],
                                        eye_bf[:])
                    # GpSimd cannot read PSUM: DVE drains the transposes,
                    # gpsimd mirrors the shared pred rows SBUF->SBUF.
                    nc.vector.tensor_copy(comb_A[0:U, vsl], tr_p[:])
                    nc.gpsimd.tensor_copy(comb_B[0:U, vsl], comb_A[0:U, vsl])
                    nc.vector.tensor_copy(comb_A[ENC_BASE:P, vsl],
                                          tr_e[0:P - ENC_BASE, :])
                    nc.vector.tensor_copy(
                        comb_B[ENC_BASE:ENC_BASE + TPC - ENC_WIN_B, vsl],
                        tr_e[ENC_WIN_B:TPC, :])

                for j in range(NVC):
                    projT = psA.tile([P, TU], f32)
                    for c in range(DC):
                        nc.tensor.matmul(projT[:, 0:TPC],
                                         wt[j][:, c, :],
                                         enc_bf[:, c, :],
                                         start=(c == 0), stop=(c == DC - 1))
                    for c in range(DC):
                        nc.tensor.matmul(projT[:, TPC:TU],
                                         wt[j][:, DC + c, :],
                                         pred_bf[:, c, :],
                                         start=(c == 0), stop=(c == DC - 1))
                    # b enters the exp via the ACT bias operand (v is on
                    # partitions here, so bT[:, j] is the right column)
                    nc.scalar.activation(expT_sb[:, j, 0:TPC],
                                         projT[:, 0:TPC], Act.Exp)
                    nc.scalar.activation(expT_sb[:, j, TPC:TU],
                                         projT[:, TPC:TU], Act.Exp,
                                         bias=bT_sb[:, j:j + 1])
                    nc.vector.tensor_copy(proj_sb[:, j, :], projT[:])
                    if j >= 1:
                        emit_tail(j - 1)
                emit_tail(NVC - 1)

            # ---- main loop ---------------------------------------------
            # x tile = one one-hot matmul pass over comb (emits e+p+b);
            # the -lse lands in the PSUM->SBUF cast via the per-partition
            # scalar operand, so the matmuls depend only on hot + comb.
            with tc.tile_pool(name="psX", bufs=4, space=PSUM) as psX:
                for k in range(NT):
                    r0 = k * P
                    rows = min(P, ROWS - r0)
                    comb = comb_A if k < A_TILES else comb_B
                    x_ps = psX.tile([P, V], f32, tag="x")
                    for half in range(2):
                        sl = slice(half * 512, (half + 1) * 512)
                        nc.tensor.matmul(x_ps[:, sl], hot_sb[:, k, :],
                                         comb[:, sl], start=True, stop=True)
                    o = outs.tile([P, V], bf16)
                    if k % 2 == 0:
                        nc.vector.tensor_copy(o[:rows], x_ps[:rows])
                    else:
                        nc.scalar.activation(o[:rows], x_ps[:rows],
                                             Act.Identity)
                    eng = nc.sync if k % 2 == 0 else nc.gpsimd
                    eng.dma_start(out=out.ap()[r0:r0 + rows, :], in_=o[:rows])

    nc.compile()
    return nc


def kernel(enc_out, pred_out, W, b):
    global LAST_RESULT
    enc_out = np.asarray(enc_out, dtype=np.float32)
    pred_out = np.asarray(pred_out, dtype=np.float32)
    W = np.asarray(W, dtype=np.float32)
    b = np.asarray(b, dtype=np.float32)

    if "nc" not in _CACHE:
        _CACHE["nc"] = _build_program()
    nc = _CACHE["nc"]

    import ml_dtypes
    wT = np.ascontiguousarray(W.T)                              # [2D, V]
    # wTc[j, p, half*4+c, v'] = wT[512*half + 4p + c, 128j + v']
    wTc = np.ascontiguousarray(
        wT.reshape(2, P, DC, NPIECE, P)
          .transpose(3, 1, 0, 2, 4)
          .reshape(NPIECE, P, 2 * DC, P)).astype(ml_dtypes.bfloat16)
    bB = np.ascontiguousarray(b.reshape(1, V)).astype(ml_dtypes.bfloat16)
    bT = np.ascontiguousarray(b.reshape(NVC, P).T)              # [128, 8]
    enc_flat = enc_out.reshape(BT, D)                           # [800, 512]

    in_maps = []
    for c in range(N_CORES):
        bt0 = c * TPC
        b_idx = bt0 // T
        encCh = np.ascontiguousarray(
            enc_flat[bt0:bt0 + TPC].T.reshape(P, DC, TPC)).astype(
                ml_dtypes.bfloat16)
        predCh = np.ascontiguousarray(
            pred_out[b_idx].T.reshape(P, DC, U)).astype(ml_dtypes.bfloat16)
        in_maps.append({
            "encC": encCh,
            "predC": predCh,
            "wTc": wTc,
            "biasB": bB,
            "biasT": bT,
        })

    res = run_bass_kernel_spmd(nc, in_maps, core_ids=list(range(N_CORES)),
                               trace=TRACE)
    LAST_RESULT = res
    full = np.concatenate(
        [np.asarray(r["out"]) for r in res.results], axis=0)
    return full.astype(np.float32).reshape(B, T, U, V)
